# revision 2
# baseline (speedup 1.0000x reference)
"""Multi-head self-attention kernel for Trainium2 (Bass/Tile), 8 NeuronCores.

Problem (hardcoded): x [4096, 512] f32; per-head Linear(512, 512) with weight
W[h] [512, 512] (torch [out, in]) and bias b[h] [512]; h = x @ W[h].T + b[h];
scores = h @ h.T; attn = softmax(scores, -1); out_h = attn @ x; final output
is the head-major concat [4096, 8*512].

Sharding: head parallel — core c computes head c entirely (its own W/b slice
arrives via the per-core input map); the host concatenates the 8 per-head
[4096, 512] outputs along the feature axis.

Numerics: matmuls run as float32r (TF32-like, 11-bit mantissa, fp32
accumulate in PSUM) which is ~4x the fp32 matmul rate on the PE array.
Softmax uses bias = -||h_q||^2: the Gram-matrix diagonal is the row max here
(h rows are near-orthogonal, offdiag |scores| << diag), and any per-row bias
deviation is divided out exactly by the final normalization.
"""
import numpy as np
from contextlib import ExitStack

N, D, H = 4096, 512, 8
P = 128
NB = N // P          # 32 n-blocks
DB = D // P          # 4 d-chunks
MC = N // 512        # 8 m-chunks of 512
N_CORES = 8

_CACHE = {}


def _build():
    from concourse import bacc, tile, mybir, masks

    dt = mybir.dt
    f32, f32r = dt.float32, dt.float32r
    AF = mybir.ActivationFunctionType
    ALU = mybir.AluOpType

    nc = bacc.Bacc("TRN2", target_bir_lowering=False, debug=False)

    X = nc.dram_tensor("x", [N, D], f32, kind="ExternalInput")
    W = nc.dram_tensor("w", [D, D], f32, kind="ExternalInput")
    B = nc.dram_tensor("b", [D, 1], f32, kind="ExternalInput")
    OUT = nc.dram_tensor("out", [N, D], f32, kind="ExternalOutput")

    with tile.TileContext(nc) as tc, ExitStack() as ctx:
        # ---- persistent pools -------------------------------------------
        const_pool = ctx.enter_context(tc.tile_pool(name="const", bufs=1))
        x_pool = ctx.enter_context(tc.tile_pool(name="x", bufs=1))
        hT_pool = ctx.enter_context(tc.tile_pool(name="hT", bufs=1))

        ident = const_pool.tile([P, P], f32)
        masks.make_identity(nc, ident[:])
        ident_r = const_pool.tile([P, P], f32r)
        nc.vector.tensor_copy(ident_r[:], ident[:])
        ones = const_pool.tile([P, 1], f32)
        nc.gpsimd.memset(ones[:], 1.0)
        ones_r = const_pool.tile([P, 1], f32r)
        nc.vector.tensor_copy(ones_r[:], ones[:])
        b_sb = const_pool.tile([P, DB], f32)
        for ob in range(DB):
            nc.sync.dma_start(b_sb[:, ob : ob + 1], B.ap()[ob * P : (ob + 1) * P, :])
        bias_cols = const_pool.tile([P, NB], f32)

        # x in natural layout: x_sb[p, j, d] = x[j*128 + p, d]
        x_sb = x_pool.tile([P, NB, D], f32r)
        for j in range(NB):
            nc.sync.dma_start(
                x_sb[:, j, :], X.ap()[j * P : (j + 1) * P, :].bitcast(f32r)
            )

        # hT[p, dc, n] = h[n, dc*128 + p]
        hT = hT_pool.tile([P, DB, N], f32r)

        # ---- phase 1: hT = (x @ W.T + b).T ------------------------------
        with ExitStack() as p1:
            w_pool = p1.enter_context(tc.tile_pool(name="wp", bufs=1))
            xT_pool = p1.enter_context(tc.tile_pool(name="xTp", bufs=2))
            tr_ps_pool = p1.enter_context(
                tc.tile_pool(name="p1tr", bufs=3, space="PSUM")
            )
            h_ps_pool = p1.enter_context(
                tc.tile_pool(name="p1h", bufs=4, space="PSUM")
            )

            w_sb = w_pool.tile([P, DB, D], f32r)
            for ob in range(DB):
                nc.sync.dma_start(
                    w_sb[:, ob, :], W.ap()[ob * P : (ob + 1) * P, :].bitcast(f32r)
                )
            # wT[p, dc, o] = W[o, dc*128 + p]
            wT = w_pool.tile([P, DB, D], f32r)
            for ob in range(DB):
                tp = tr_ps_pool.tile([P, DB, P], f32r, tag="tr")
                for dc in range(DB):
                    nc.tensor.transpose(
                        tp[:, dc, :],
                        w_sb[:, ob, dc * P : (dc + 1) * P],
                        ident_r[:],
                    )
                # tp[p, dc, q] = W[ob*128+q, dc*128+p] -> wT[p, dc, ob*128+q]
                nc.vector.tensor_copy(wT[:, :, ob * P : (ob + 1) * P], tp[:])

            for nc512 in range(MC):
                xT = xT_pool.tile([P, DB, 512], f32r, tag="xT")
                for j2 in range(4):
                    j = nc512 * 4 + j2
                    tp = tr_ps_pool.tile([P, DB, P], f32r, tag="tr")
                    for dc in range(DB):
                        nc.tensor.transpose(
                            tp[:, dc, :],
                            x_sb[:, j, dc * P : (dc + 1) * P],
                            ident_r[:],
                        )
                    nc.vector.tensor_copy(xT[:, :, j2 * P : (j2 + 1) * P], tp[:])
                for ob in range(DB):
                    hp = h_ps_pool.tile([P, 512], f32, tag="h")
                    for dc in range(DB):
                        nc.tensor.matmul(
                            hp[:],
                            wT[:, dc, ob * P : (ob + 1) * P],
                            xT[:, dc, :],
                            start=(dc == 0),
                            stop=(dc == DB - 1),
                        )
                    nc.scalar.activation(
                        hT[:, ob, nc512 * 512 : (nc512 + 1) * 512],
                        hp[:],
                        AF.Identity,
                        bias=b_sb[:, ob : ob + 1],
                        scale=1.0,
                    )

        # ---- phase 1.5: bias_cols[:, Q] = -||h_q||^2 --------------------
        with ExitStack() as p15:
            sq_pool = p15.enter_context(tc.tile_pool(name="sq", bufs=2))
            ss_pool = p15.enter_context(tc.tile_pool(name="ss", bufs=1))
            n2_pool = p15.enter_context(tc.tile_pool(name="n2", bufs=1))
            n2_ps_pool = p15.enter_context(
                tc.tile_pool(name="n2ps", bufs=2, space="PSUM")
            )

            sqsum = ss_pool.tile([P, N], f32r)
            nc.scalar.activation(sqsum[:], hT[:, 0, :], AF.Square)
            for dc in range(1, DB):
                sq = sq_pool.tile([P, N], f32r, tag="sq")
                nc.scalar.activation(sq[:], hT[:, dc, :], AF.Square)
                nc.vector.tensor_add(sqsum[:], sqsum[:], sq[:])
            n2row = n2_pool.tile([1, N], f32)
            for mc in range(MC):
                ps = n2_ps_pool.tile([1, 512], f32, tag="n2")
                nc.tensor.matmul(
                    ps[:],
                    ones_r[:, 0:1],
                    sqsum[:, mc * 512 : (mc + 1) * 512],
                    start=True,
                    stop=True,
                )
                nc.vector.tensor_copy(n2row[0:1, mc * 512 : (mc + 1) * 512], ps[:])
            n2resh = n2_pool.tile([NB, P], f32)
            nc.sync.dma_start(n2resh[:], n2row[:])
            psT = n2_ps_pool.tile([P, NB], f32, tag="n2t")
            nc.tensor.transpose(psT[:], n2resh[:], ident[0:NB, 0:NB])
            nc.vector.tensor_scalar(bias_cols[:], psT[:], -1.0, None, op0=ALU.mult)

        # ---- phase 2: per q-block scores/softmax/AV ---------------------
        E_pool = ctx.enter_context(tc.tile_pool(name="E", bufs=2))
        ET_pool = ctx.enter_context(tc.tile_pool(name="ET", bufs=1))
        st_pool = ctx.enter_context(tc.tile_pool(name="st", bufs=3))
        out_pool = ctx.enter_context(tc.tile_pool(name="outp", bufs=3))
        sc_ps_pool = ctx.enter_context(tc.tile_pool(name="scps", bufs=3, space="PSUM"))
        tr_ps_pool2 = ctx.enter_context(tc.tile_pool(name="trps", bufs=2, space="PSUM"))
        o_ps_pool = ctx.enter_context(tc.tile_pool(name="ops", bufs=2, space="PSUM"))

        for Q in range(NB):
            E_t = E_pool.tile([P, N], f32r, tag="E")
            acc = st_pool.tile([P, MC], f32, tag="acc")
            for mc in range(MC):
                s_ps = sc_ps_pool.tile([P, 512], f32, tag="s")
                for dc in range(DB):
                    nc.tensor.matmul(
                        s_ps[:],
                        hT[:, dc, Q * P : (Q + 1) * P],
                        hT[:, dc, mc * 512 : (mc + 1) * 512],
                        start=(dc == 0),
                        stop=(dc == DB - 1),
                    )
                nc.scalar.activation(
                    E_t[:, mc * 512 : (mc + 1) * 512],
                    s_ps[:],
                    AF.Exp,
                    bias=bias_cols[:, Q : Q + 1],
                    scale=1.0,
                    accum_out=acc[:, mc : mc + 1],
                )
            rowsum = st_pool.tile([P, 1], f32, tag="rs")
            nc.vector.tensor_reduce(
                rowsum[:], acc[:], axis=mybir.AxisListType.XYZW, op=ALU.add
            )
            recip = st_pool.tile([P, 1], f32, tag="rcp")
            nc.vector.reciprocal(recip[:], rowsum[:])

            ET_t = ET_pool.tile([P, N], f32r, tag="ET")
            for g in range(MC):
                t_ps = tr_ps_pool2.tile([P, 512], f32r, tag="t")
                for t in range(4):
                    mi = g * 4 + t
                    nc.tensor.transpose(
                        t_ps[:, t * P : (t + 1) * P],
                        E_t[:, mi * P : (mi + 1) * P],
                        ident_r[:],
                    )
                nc.vector.tensor_copy(ET_t[:, g * 512 : (g + 1) * 512], t_ps[:])

            o_ps = o_ps_pool.tile([P, D], f32, tag="o")
            for ki in range(NB):
                nc.tensor.matmul(
                    o_ps[:],
                    ET_t[:, ki * P : (ki + 1) * P],
                    x_sb[:, ki, :],
                    start=(ki == 0),
                    stop=(ki == NB - 1),
                )
            out_sb = out_pool.tile([P, D], f32, tag="out")
            nc.vector.tensor_scalar(
                out_sb[:], o_ps[:], recip[:, 0:1], None, op0=ALU.mult
            )
            nc.sync.dma_start(OUT.ap()[Q * P : (Q + 1) * P, :], out_sb[:])

    nc.compile()
    return nc


def _get_nc():
    if "nc" not in _CACHE:
        _CACHE["nc"] = _build()
    return _CACHE["nc"]


def kernel(x_resting: np.ndarray, W: np.ndarray, b: np.ndarray) -> np.ndarray:
    from concourse.bass_utils import run_bass_kernel_spmd

    nc = _get_nc()
    in_maps = [
        {
            "x": np.ascontiguousarray(x_resting, dtype=np.float32),
            "w": np.ascontiguousarray(W[c], dtype=np.float32),
            "b": np.ascontiguousarray(b[c].reshape(D, 1), dtype=np.float32),
        }
        for c in range(N_CORES)
    ]
    res = run_bass_kernel_spmd(nc, in_maps, list(range(N_CORES)))
    return np.concatenate([res.results[c]["out"] for c in range(N_CORES)], axis=1)
